# revision 31
# baseline (speedup 1.0000x reference)
"""Trainium2 Bass kernel for nn_MultiHeadAttention_44908178047033.

T5-style MHA (relative-position bias, bidirectional) over
B=2, L=2048, D=768, H=12, DK=64.

Sharding: 8 cores = 2 batches x 4 head-groups (3 heads each).
Each core computes Q/K/V projections for its (batch, 3 heads), fused
transposed-orientation attention (scores kept as S^T [k, q] so the
softmax denominator and the PV contraction both run as PE matmuls
without transposing the probability matrix), and a partial output
projection. Host sums the 4 per-head-group partials per batch.

v3 perf structure (from trace analysis of the f32r baseline and v2):
- everything bf16: halves input DMA, runs all matmuls at bf16 rate
- single ACT table preload (natural_log_exp_and_others) so the Ln/Exp
  softmax normalization never swaps activation tables mid-kernel
- q-half-major loop with sequential heads: only 2 PV accumulator banks
  and 2 double-buffered score tiles are live, leaving 2 PSUM banks for
  interleaved filler matmuls
- the PE HAM clock gate re-throttles to 1.2 GHz after any ~3.4us idle
  window and only re-warms after ~3.4us of continuous work, so the V
  projection is interleaved into q-half 0's attention and the output
  projection of q-half 0 into q-half 1's attention: the PE instruction
  queue never drains at phase transitions and stays at 2.4 GHz

Relative-position bias: the T5 bias f(k-q) is constant for |k-q| >= 128
(log-bucketing saturates), so
  exp(s + f) = exp(s + cm)            for k-q <= -128  (ACT bias, free)
             = exp(s + cp)            for k-q >= +128  (ACT bias, free)
             = exp(s + cm + (f - cm)) for |k-q| < 128  (DVE add from a
               host-precomputed per-partition shifted Toeplitz table,
               read with a negative free-dim stride)
"""

import math
import sys
import threading

import numpy as np

sys.path.insert(0, "/opt/trn_rl_repo")

B, L, D = 2, 2048, 768
H, DK = 12, 64
NUM_BUCKETS, MAX_DIST = 32, 128
HP = 3            # heads per core
HD = HP * DK      # 192 cols per head-group
NCORES = 8
KC = 16           # key chunks of 128
CCH = 6           # contraction chunks of 128 over D

_cache = {}
_lock = threading.Lock()


def _np_bucket(d):
    rel = d
    ret = np.zeros_like(rel)
    n = -rel
    nb = NUM_BUCKETS // 2
    ret = ret + (n < 0).astype(np.int32) * nb
    n = np.abs(n)
    mx = nb // 2
    is_small = n < mx
    n_safe = np.maximum(n, 1).astype(np.float32)
    vl = mx + (
        np.log(n_safe / mx) / math.log(MAX_DIST / mx) * (nb - mx)
    ).astype(np.int32)
    vl = np.minimum(vl, nb - 1)
    return ret + np.where(is_small, n, vl)


def _build_program():
    import concourse.bacc as bacc
    import concourse.bass as bass
    import concourse.mybir as mybir
    import concourse.tile as tile
    from concourse.hw_specs import get_activation_tables

    dt = mybir.dt
    f32, bf16 = dt.float32, dt.bfloat16
    Exp, Ln = mybir.ActivationFunctionType.Exp, mybir.ActivationFunctionType.Ln

    act_sets = list(get_activation_tables("gen3").keys())
    nle_id = act_sets.index("natural_log_exp_and_others")

    nc = bacc.Bacc("TRN2", target_bir_lowering=False, debug=False,
                   num_devices=NCORES)

    qT_d = nc.dram_tensor("qT", [D, L], bf16, kind="ExternalInput").ap()
    kvT_d = nc.dram_tensor("kvT", [D, L], bf16, kind="ExternalInput").ap()
    wqa_d = nc.dram_tensor("wqa", [D, 128], bf16, kind="ExternalInput").ap()
    wka_d = nc.dram_tensor("wka", [D, 128], bf16, kind="ExternalInput").ap()
    wqb_d = nc.dram_tensor("wqb", [D, 64], bf16, kind="ExternalInput").ap()
    wkb_d = nc.dram_tensor("wkb", [D, 64], bf16, kind="ExternalInput").ap()
    wv_d = nc.dram_tensor("wv", [D, HD], bf16, kind="ExternalInput").ap()
    wo_d = nc.dram_tensor("wo", [64, HP, D], bf16, kind="ExternalInput").ap()
    sh_d = nc.dram_tensor("sh", [HP, 128, 383], f32, kind="ExternalInput").ap()
    msk_d = nc.dram_tensor("msk", [128, KC], f32, kind="ExternalInput").ap()
    cm_d = nc.dram_tensor("cm", [128, HP], f32, kind="ExternalInput").ap()
    cp_d = nc.dram_tensor("cp", [128, HP], f32, kind="ExternalInput").ap()
    out_d = nc.dram_tensor("out_p", [L, D], bf16, kind="ExternalOutput").ap()

    with tile.TileContext(nc) as tc:
        with (
            tc.tile_pool(name="const", bufs=1) as cpool,
            tc.tile_pool(name="p", bufs=4) as ppool,
            tc.tile_pool(name="o", bufs=2) as opool,
            tc.tile_pool(name="nrm", bufs=4) as npool,
            tc.tile_pool(name="sp", bufs=2, space="PSUM") as sp,
            tc.tile_pool(name="pp", bufs=4, space="PSUM") as pp,
        ):
            # ---- persistent SBUF ----
            wqa = cpool.tile([128, CCH, 128], bf16, tag="wqa")
            wka = cpool.tile([128, CCH, 128], bf16, tag="wka")
            wqb = cpool.tile([128, CCH, 64], bf16, tag="wqb")
            wkb = cpool.tile([128, CCH, 64], bf16, tag="wkb")
            wv = cpool.tile([128, CCH, HD], bf16, tag="wv")
            wo = cpool.tile([64, HP, D], bf16, tag="wo")
            sh = cpool.tile([128, HP, 383], f32, tag="sh")
            msk = cpool.tile([128, KC], f32, tag="msk")
            cmc = cpool.tile([128, HP], f32, tag="cmc")
            cpc = cpool.tile([128, HP], f32, tag="cpc")
            # per-contraction-chunk tiles so each projection matmul only
            # waits on its own chunk's DMA, not the whole activation load
            qT = [cpool.tile([128, L], bf16, tag=f"qT{c}", name=f"qT{c}")
                  for c in range(CCH)]
            kvT = [cpool.tile([128, L], bf16, tag=f"kvT{c}", name=f"kvT{c}")
                   for c in range(CCH)]
            oacc = cpool.tile([128, 8, D], f32, tag="oacc")
            # heads 0,1 stacked on partitions 0-63 / 64-127
            QTa = cpool.tile([128, L], bf16, tag="QTa")
            KTa = cpool.tile([128, L], bf16, tag="KTa")
            # head 2: K on partitions 64-127 (straight from its col-tiled
            # projection), Q on 0-63 then DMA-duplicated to 64-127 so both
            # score operands live on the same partition half
            QTb = cpool.tile([128, L], bf16, tag="QTb")
            KTb = cpool.tile([128, L], bf16, tag="KTb")
            Vg = cpool.tile([128, KC, HP, 65], bf16, tag="Vg")
            AT = cpool.tile([64, HP, L], bf16, tag="AT")
            ones = cpool.tile([1, 64], bf16, tag="ones")
            nc.vector.memset(ones[:], 1.0)

            # single activation-table load covering both Exp and Ln; the
            # compile-time pass then sees every activation's table resident
            nc.scalar.add_instruction(mybir.InstLoadActFuncSet(
                name=nc.get_next_instruction_name(), ins=[], outs=[],
                act_func_set_id=nle_id))

            # ---- loads; inputs split across both HWDGE queues (kvT on
            # sync, qT on scalar behind the small weights) so the two
            # streams land in parallel and the projections start early ----
            nc.scalar.dma_start(out=wka[:], in_=wka_d.rearrange("(c p) n -> p c n", p=128))
            nc.scalar.dma_start(out=wqa[:], in_=wqa_d.rearrange("(c p) n -> p c n", p=128))
            nc.scalar.dma_start(out=wv[:], in_=wv_d.rearrange("(c p) n -> p c n", p=128))
            nc.scalar.dma_start(out=wqb[:], in_=wqb_d.rearrange("(c p) n -> p c n", p=128))
            nc.scalar.dma_start(out=wkb[:], in_=wkb_d.rearrange("(c p) n -> p c n", p=128))
            nc.scalar.dma_start(out=sh[:], in_=sh_d.rearrange("h p y -> p h y"))
            nc.scalar.dma_start(out=msk[:], in_=msk_d)
            nc.scalar.dma_start(out=cmc[:], in_=cm_d)
            nc.scalar.dma_start(out=cpc[:], in_=cp_d)
            nc.scalar.dma_start(out=wo[:], in_=wo_d)
            qT_r = qT_d.rearrange("(c p) n -> p c n", p=128)
            kvT_r = kvT_d.rearrange("(c p) n -> p c n", p=128)
            for c in range(CCH):
                nc.sync.dma_start(out=kvT[c][:], in_=kvT_r[:, c, :])
            for c in range(CCH):
                nc.scalar.dma_start(out=qT[c][:], in_=qT_r[:, c, :])

            # ---- Q/K projections (heads 0,1; head 2 is deferred into
            # q-half 0's attention phases as PE filler) ----
            for w_in, x_in, dst in ((wka, kvT, KTa), (wqa, qT, QTa)):
                for n in range(4):
                    nsl = slice(512 * n, 512 * n + 512)
                    ps = pp.tile([128, 512], f32, tag="pp", name=f"ps{dst.name}_{n}")
                    for c in range(CCH):
                        nc.tensor.matmul(
                            ps[:], lhsT=w_in[:, c, :], rhs=x_in[c][:, nsl],
                            start=(c == 0), stop=(c == CCH - 1),
                        )
                    nc.vector.tensor_copy(dst[:, nsl], ps[:])

            def qk2_proj(n):
                """head 2's Q and K projection n-chunk, col-tiled in one
                pass (Q on array cols 0-63, K on 64-127), separate PSUM
                banks so the accumulation groups' has_written clears stay
                apart."""
                nsl = slice(512 * n, 512 * n + 512)
                psq = pp.tile([128, 512], f32, tag="pp", name=f"psbq_{n}")
                psk = pp.tile([128, 512], f32, tag="pp", name=f"psbk_{n}")
                for c in range(CCH):
                    nc.tensor.matmul(
                        psq[0:64, :], lhsT=wqb[:, c, :], rhs=qT[c][:, nsl],
                        start=(c == 0), stop=(c == CCH - 1),
                    )
                    nc.tensor.matmul(
                        psk[64:128, :], lhsT=wkb[:, c, :], rhs=kvT[c][:, nsl],
                        start=(c == 0), stop=(c == CCH - 1),
                        tile_position=(0, 64),
                    )
                nc.vector.tensor_copy(QTb[0:64, nsl], psq[0:64, :])
                nc.vector.tensor_copy(KTb[64:128, nsl], psk[64:128, :])

            # mask column of V_aug, all key chunks at once
            mrep = bass.AP(msk[:].tensor, msk[:].offset,
                           [list(msk[:].ap[0]), [1, KC], [0, HP], [1, 1]])
            nc.vector.tensor_copy(Vg[:, :, :, 64:65], mrep)

            def v_proj(kc, h):
                """V projection chunk for one head -> Vg[:, kc, h, 0:64]
                (interleaved just-in-time into that head's q-half 0
                attention as PE filler work)."""
                ps_v = pp.tile([128, 512], f32, tag="pp", name=f"psv{kc}_{h}")
                for c in range(CCH):
                    nc.tensor.matmul(
                        ps_v[:, 0:64],
                        lhsT=kvT[c][:, 128 * kc:128 * kc + 128],
                        rhs=wv[:, c, 64 * h:64 * h + 64],
                        start=(c == 0), stop=(c == CCH - 1),
                    )
                nc.vector.tensor_copy(Vg[:, kc, h, 0:64], ps_v[:, 0:64])

            def band_add(s, h, kc, ha):
                """near-diagonal bias add (in place, PSUM); s covers
                columns [ha, ha+1024)."""
                qlo = max(0, 128 * kc - 128)
                qhi = min(L, 128 * kc + 255)
                x0 = (2047 + 128 * kc - qlo) - 1793
                a = max(qlo, ha)
                b = min(qhi, ha + 1024)
                if b > a:
                    sh_ap = sh[:, h, :]
                    rev = bass.AP(
                        sh_ap.tensor, sh_ap.offset + x0 - (a - qlo),
                        [list(sh_ap.ap[0]), [-1, b - a]],
                    )
                    nc.vector.tensor_add(
                        s[:, a - ha:b - ha], s[:, a - ha:b - ha], rev)

            def exp_split(s, h, kc, ha, name):
                """exp with region-split bias: cp for q < wcp, cm after."""
                p = ppool.tile([128, 1024], bf16, tag="p", name=name)
                wcp = max(0, 128 * kc - 128)
                wl = min(max(wcp - ha, 0), 1024)
                if wl > 0:
                    nc.scalar.activation(
                        p[:, 0:wl], s[:, 0:wl], Exp,
                        bias=cpc[:, h:h + 1], scale=1.0)
                if wl < 1024:
                    nc.scalar.activation(
                        p[:, wl:1024], s[:, wl:1024], Exp,
                        bias=cmc[:, h:h + 1], scale=1.0)
                return p

            def normalize(pvs, h, qh):
                """pvs: two [65, 512] PSUM accumulators (numerator rows
                0-63, denominator row 64) -> AT[:, h, qh*1024 : +1024].
                Ln reads the denominator straight from PSUM, and 1/D is
                broadcast across partitions with a K=1 matmul (ones.T @
                inv) — much lower latency than a GpSimd broadcast."""
                pvsb = npool.tile([65, 1024], bf16, tag="pvsb",
                                  name=f"pvsb{qh}_{h}")
                for j in range(2):
                    lns = npool.tile([1, 512], f32, tag="lns",
                                     name=f"l{qh}_{h}_{j}")
                    nc.scalar.activation(lns[:], pvs[j][64:65, :], Ln)
                    nc.vector.tensor_copy(pvsb[:, 512 * j:512 * j + 512],
                                          pvs[j][:])
                    inv = npool.tile([1, 512], bf16, tag="inv",
                                     name=f"i{qh}_{h}_{j}")
                    nc.scalar.activation(inv[:], lns[:], Exp, scale=-1.0)
                    invb = pp.tile([64, 512], f32, tag="pp",
                                   name=f"ib{qh}_{h}_{j}")
                    nc.tensor.matmul(invb[:], lhsT=ones[:], rhs=inv[:],
                                     start=True, stop=True)
                    qsl = slice(1024 * qh + 512 * j,
                                1024 * qh + 512 * j + 512)
                    nc.vector.tensor_mul(
                        AT[:, h, qsl],
                        pvsb[0:64, 512 * j:512 * j + 512], invb[:])

            _ostate = {}

            def out_proj_unit(qc, nlo):
                """one output-projection unit: 3 accumulating matmuls for
                query rows [128*qc, +128), output cols [nlo, nlo+nw); the
                second unit of a row chunk completes the tile and DMAs it."""
                nw = 512 if nlo == 0 else 256
                if nlo == 0:
                    _ostate[qc] = opool.tile([128, D], bf16, tag="o",
                                             name=f"o{qc}")
                o = _ostate[qc]
                ps_o = pp.tile([128, 512], f32, tag="pp", name=f"po{qc}_{nlo}")
                for h in range(HP):
                    nc.tensor.matmul(
                        ps_o[:, 0:nw],
                        lhsT=AT[:, h, 128 * qc:128 * qc + 128],
                        rhs=wo[:, h, nlo:nlo + nw],
                        start=(h == 0), stop=(h == HP - 1),
                    )
                nc.vector.tensor_copy(o[:, nlo:nlo + nw], ps_o[:, 0:nw])
                if nlo != 0:
                    nc.sync.dma_start(
                        out=out_d[128 * qc:128 * qc + 128, :], in_=o[:])

            def out_proj_a(qc, nlo, heads):
                """q-half-1 output projection, stage A: partial sum over
                `heads` into the SBUF accumulator (interleaved as PE filler
                before the last head's attention finishes)."""
                nw = 512 if nlo == 0 else 256
                ps_o = pp.tile([128, 512], f32, tag="pp",
                               name=f"pa{qc}_{nlo}")
                for i, h in enumerate(heads):
                    nc.tensor.matmul(
                        ps_o[:, 0:nw],
                        lhsT=AT[:, h, 128 * qc:128 * qc + 128],
                        rhs=wo[:, h, nlo:nlo + nw],
                        start=(i == 0), stop=(i == len(heads) - 1),
                    )
                nc.vector.tensor_copy(oacc[:, qc - 8, nlo:nlo + nw],
                                      ps_o[:, 0:nw])

            def out_proj_b(qc, nlo, h):
                """stage B: last head's contribution + accumulator -> out."""
                nw = 512 if nlo == 0 else 256
                if nlo == 0:
                    _ostate[qc] = opool.tile([128, D], bf16, tag="o",
                                             name=f"o{qc}")
                o = _ostate[qc]
                ps_o = pp.tile([128, 512], f32, tag="pp",
                               name=f"pb{qc}_{nlo}")
                nc.tensor.matmul(ps_o[:, 0:nw],
                                 lhsT=AT[:, h, 128 * qc:128 * qc + 128],
                                 rhs=wo[:, h, nlo:nlo + nw],
                                 start=True, stop=True)
                nc.vector.tensor_add(o[:, nlo:nlo + nw],
                                     oacc[:, qc - 8, nlo:nlo + nw],
                                     ps_o[:, 0:nw])
                if nlo != 0:
                    nc.sync.dma_start(
                        out=out_d[128 * qc:128 * qc + 128, :], in_=o[:])

            _hb = [0]

            def heartbeat(tile=None):
                """dummy matmul into a scratch PSUM slot; its only purpose
                is keeping the PE's HAM activity monitor from re-throttling
                the clock to 1.2 GHz during ACT-bound stretches. Passing a
                shared tile serializes consecutive heartbeats (write-after-
                write), spreading them across a known stall instead of
                letting the scheduler fire them all at once."""
                _hb[0] += 1
                hb = tile if tile is not None else pp.tile(
                    [128, 512], f32, tag="pp", name=f"hb{_hb[0]}")
                nc.tensor.matmul(hb[:], lhsT=wqa[:, 0, :],
                                 rhs=KTa[:, 0:512], start=True, stop=True)

            def head_ops(h):
                if h == 0:
                    return (lambda kc: KTa[0:64, 128 * kc:128 * kc + 128],
                            lambda lo: QTa[0:64, lo:lo + 512])
                if h == 1:
                    return (lambda kc: KTa[64:128, 128 * kc:128 * kc + 128],
                            lambda lo: QTa[64:128, lo:lo + 512])
                return (lambda kc: KTb[64:128, 128 * kc:128 * kc + 128],
                        lambda lo: QTb[64:128, lo:lo + 512])

            # ---- fused attention, transposed orientation, q-half major ----
            # PE filler schedule keeps the tensor engine's queue from
            # draining (and its HAM clock gate from re-throttling):
            #   qh0/h: that head's V-projection chunks just-in-time, plus
            #          head 2's deferred Q/K projection split over h0/h1
            #   qh1 runs heads (2, 0, 1): q-half 0's output projection
            #          fills h2 and h0; q-half 1's own output projection
            #          runs 2-stage, heads {2,0} filling the h1 phase and
            #          only head 1's single matmul per unit in the tail
            qh0_units = iter([(qc, nlo) for qc in range(8)
                              for nlo in (0, 512)])
            qh1a_units = iter([(qc, nlo) for qc in range(8, 16)
                               for nlo in (0, 512)])
            for qh in range(2):
                ha = 1024 * qh
                heads = (0, 1, 2) if qh == 0 else (2, 0, 1)
                for hi, h in enumerate(heads):
                    kslice, qslice = head_ops(h)
                    pvs = [pp.tile([65, 512], f32, tag="pp",
                                   name=f"pv{qh}_{h}_{j}") for j in range(2)]
                    if qh == 0:
                        v_proj(0, h)
                        v_proj(1, h)
                    for kc in range(KC):
                        s = sp.tile([128, 1024], f32, tag="sp",
                                    name=f"s{qh}_{h}_{kc}")
                        for jj in range(2):
                            nc.tensor.matmul(
                                s[:, 512 * jj:512 * jj + 512],
                                lhsT=kslice(kc), rhs=qslice(ha + 512 * jj),
                                start=True, stop=True)
                        band_add(s, h, kc, ha)
                        p = exp_split(s, h, kc, ha, f"p{qh}_{h}_{kc}")
                        for jj in range(2):
                            nc.tensor.matmul(
                                pvs[jj][:],
                                lhsT=Vg[:, kc, h, :],
                                rhs=p[:, 512 * jj:512 * jj + 512],
                                start=(kc == 0), stop=(kc == KC - 1))
                        if qh == 0 and kc + 2 < KC:
                            v_proj(kc + 2, h)
                        if qh == 0 and hi < 2 and kc % 8 == 4:
                            qk2_proj(2 * hi + kc // 8)
                        if qh == 1 and hi < 2:
                            if kc % 2 == 0:
                                unit = next(qh0_units, None)
                                if unit is not None:
                                    out_proj_unit(*unit)
                            else:
                                heartbeat()
                        if qh == 1 and hi == 2:
                            unit = next(qh1a_units, None)
                            if unit is not None:
                                out_proj_a(*unit, heads=(2, 0))
                    normalize(pvs, h, qh)
                    if qh == 0 and h == 1:
                        # head-2 scores contract on partitions 64-127; its
                        # Q was projected onto 0-63 during the h0/h1 phases
                        nc.sync.dma_start(out=QTb[64:128, :], in_=QTb[0:64, :])

            # ---- tail: last head's output-projection contribution ----
            # (a serialized heartbeat chain bridges the final normalize's
            # latency so the tail matmuls run at full clock)
            hbt = pp.tile([128, 512], f32, tag="pp", name="hbt")
            for _ in range(10):
                heartbeat(hbt)
            for qc in range(8, L // 128):
                out_proj_b(qc, 0, 1)
                out_proj_b(qc, 512, 1)

    nc.compile()
    return nc


def _get_program():
    with _lock:
        if "nc" not in _cache:
            _cache["nc"] = _build_program()
        return _cache["nc"]


def _host_prep(core, query, key_value, key_padding_mask, Wq, Wk, Wv, Wo, rel_emb):
    import ml_dtypes

    bf16 = ml_dtypes.bfloat16
    b, g = core // 4, core % 4
    mask = key_padding_mask[b].astype(np.float32)
    kv = key_value[b] * mask[:, None]
    qT = np.ascontiguousarray(query[b].T).astype(bf16)
    kvT = np.ascontiguousarray(kv.T).astype(bf16)
    sl = slice(HD * g, HD * (g + 1))
    wq = np.ascontiguousarray(Wq[:, sl])
    wk = np.ascontiguousarray(Wk[:, sl]) * np.float32(DK ** -0.5)
    wv = np.ascontiguousarray(Wv[:, sl]).astype(bf16)
    wo = np.ascontiguousarray(
        Wo[sl].reshape(HP, 64, D).transpose(1, 0, 2)).astype(bf16)

    d = np.arange(-2047, 2048)
    buckets = _np_bucket(d)
    heads = [HP * g + i for i in range(HP)]
    t = rel_emb[buckets][:, heads].astype(np.float32)  # [4095, HP]
    cm = t[0]
    cp = t[-1]
    # sh[h, p, y] = t[y + 1793 + p, h] - cm[h]
    p_i = np.arange(128)[:, None]
    y_i = np.arange(383)[None, :]
    sh = np.ascontiguousarray(
        (t[y_i + 1793 + p_i] - cm[None, None, :]).transpose(2, 0, 1))
    msk = np.ascontiguousarray(mask.reshape(KC, 128).T)
    cmc = np.ascontiguousarray(np.broadcast_to(cm[None, :], (128, HP)))
    cpc = np.ascontiguousarray(np.broadcast_to(cp[None, :], (128, HP)))
    return {
        "qT": qT, "kvT": kvT,
        "wqa": np.ascontiguousarray(wq[:, 0:128]).astype(bf16),
        "wka": np.ascontiguousarray(wk[:, 0:128]).astype(bf16),
        "wqb": np.ascontiguousarray(wq[:, 128:192]).astype(bf16),
        "wkb": np.ascontiguousarray(wk[:, 128:192]).astype(bf16),
        "wv": wv, "wo": wo,
        "sh": sh.astype(np.float32), "msk": msk,
        "cm": cmc.astype(np.float32), "cp": cpc.astype(np.float32),
    }


def make_in_maps(**inputs):
    return [_host_prep(c, **inputs) for c in range(NCORES)]


def kernel(query, key_value, key_padding_mask, Wq, Wk, Wv, Wo, rel_emb,
           _results_hook=None, _run_kwargs=None):
    from concourse.bass_utils import run_bass_kernel_spmd

    inputs = dict(query=np.asarray(query), key_value=np.asarray(key_value),
                  key_padding_mask=np.asarray(key_padding_mask),
                  Wq=np.asarray(Wq, np.float32), Wk=np.asarray(Wk, np.float32),
                  Wv=np.asarray(Wv, np.float32), Wo=np.asarray(Wo, np.float32),
                  rel_emb=np.asarray(rel_emb, np.float32))
    nc = _get_program()
    in_maps = make_in_maps(**inputs)
    res = run_bass_kernel_spmd(nc, in_maps, core_ids=list(range(NCORES)),
                               **(_run_kwargs or {}))
    if _results_hook is not None:
        _results_hook(res)
    out = np.zeros((B, L, D), np.float32)
    for c in range(NCORES):
        out[c // 4] += res.results[c]["out_p"].astype(np.float32)
    return out


# revision 36
# speedup vs baseline: 1.1521x; 1.1521x over previous
"""Trainium2 Bass kernel for nn_MultiHeadAttention_44908178047033.

T5-style MHA (relative-position bias, bidirectional) over
B=2, L=2048, D=768, H=12, DK=64.

Sharding: 8 cores = 2 batches x 4 head-groups (3 heads each).
Each core computes Q/K/V projections for its (batch, 3 heads), fused
transposed-orientation attention (scores kept as S^T [k, q] so the
softmax denominator and the PV contraction both run as PE matmuls
without transposing the probability matrix), and a partial output
projection. Host sums the 4 per-head-group partials per batch.

v3 perf structure (from trace analysis of the f32r baseline and v2):
- everything bf16: halves input DMA, runs all matmuls at bf16 rate
- single ACT table preload (natural_log_exp_and_others) so the Ln/Exp
  softmax normalization never swaps activation tables mid-kernel
- q-half-major loop with sequential heads: only 2 PV accumulator banks
  and 2 double-buffered score tiles are live, leaving 2 PSUM banks for
  interleaved filler matmuls
- the PE HAM clock gate re-throttles to 1.2 GHz after any ~3.4us idle
  window and only re-warms after ~3.4us of continuous work, so the V
  projection is interleaved into q-half 0's attention and the output
  projection of q-half 0 into q-half 1's attention: the PE instruction
  queue never drains at phase transitions and stays at 2.4 GHz

Relative-position bias: the T5 bias f(k-q) is constant for |k-q| >= 128
(log-bucketing saturates), so
  exp(s + f) = exp(s + cm)            for k-q <= -128  (ACT bias, free)
             = exp(s + cp)            for k-q >= +128  (ACT bias, free)
             = exp(s + cm + (f - cm)) for |k-q| < 128  (DVE add from a
               host-precomputed per-partition shifted Toeplitz table,
               read with a negative free-dim stride)
"""

import math
import sys
import threading

import numpy as np

sys.path.insert(0, "/opt/trn_rl_repo")

B, L, D = 2, 2048, 768
H, DK = 12, 64
NUM_BUCKETS, MAX_DIST = 32, 128
HP = 3            # heads per core
HD = HP * DK      # 192 cols per head-group
NCORES = 8
KC = 16           # key chunks of 128
CCH = 6           # contraction chunks of 128 over D

_cache = {}
_lock = threading.Lock()


def _np_bucket(d):
    rel = d
    ret = np.zeros_like(rel)
    n = -rel
    nb = NUM_BUCKETS // 2
    ret = ret + (n < 0).astype(np.int32) * nb
    n = np.abs(n)
    mx = nb // 2
    is_small = n < mx
    n_safe = np.maximum(n, 1).astype(np.float32)
    vl = mx + (
        np.log(n_safe / mx) / math.log(MAX_DIST / mx) * (nb - mx)
    ).astype(np.int32)
    vl = np.minimum(vl, nb - 1)
    return ret + np.where(is_small, n, vl)


def _build_program():
    import concourse.bacc as bacc
    import concourse.bass as bass
    import concourse.mybir as mybir
    import concourse.tile as tile
    from concourse.hw_specs import get_activation_tables

    dt = mybir.dt
    f32, bf16 = dt.float32, dt.bfloat16
    Exp, Ln = mybir.ActivationFunctionType.Exp, mybir.ActivationFunctionType.Ln

    act_sets = list(get_activation_tables("gen3").keys())
    nle_id = act_sets.index("natural_log_exp_and_others")

    nc = bacc.Bacc("TRN2", target_bir_lowering=False, debug=False,
                   num_devices=NCORES)

    qT_d = nc.dram_tensor("qT", [D, L], bf16, kind="ExternalInput").ap()
    kvT_d = nc.dram_tensor("kvT", [D, L], bf16, kind="ExternalInput").ap()
    wqa_d = nc.dram_tensor("wqa", [D, 128], bf16, kind="ExternalInput").ap()
    wka_d = nc.dram_tensor("wka", [D, 128], bf16, kind="ExternalInput").ap()
    wqb_d = nc.dram_tensor("wqb", [D, 64], bf16, kind="ExternalInput").ap()
    wkb_d = nc.dram_tensor("wkb", [D, 64], bf16, kind="ExternalInput").ap()
    wv_d = nc.dram_tensor("wv", [D, HD], bf16, kind="ExternalInput").ap()
    wo_d = nc.dram_tensor("wo", [64, HP, D], bf16, kind="ExternalInput").ap()
    sh_d = nc.dram_tensor("sh", [HP, 128, 383], f32, kind="ExternalInput").ap()
    msk_d = nc.dram_tensor("msk", [128, KC], f32, kind="ExternalInput").ap()
    cm_d = nc.dram_tensor("cm", [128, HP], f32, kind="ExternalInput").ap()
    cp_d = nc.dram_tensor("cp", [128, HP], f32, kind="ExternalInput").ap()
    out_d = nc.dram_tensor("out_p", [L, D], bf16, kind="ExternalOutput").ap()

    with tile.TileContext(nc) as tc:
        with (
            tc.tile_pool(name="const", bufs=1) as cpool,
            tc.tile_pool(name="p", bufs=4) as ppool,
            tc.tile_pool(name="o", bufs=2) as opool,
            tc.tile_pool(name="nrm", bufs=4) as npool,
            tc.tile_pool(name="sp", bufs=2, space="PSUM") as sp,
            tc.tile_pool(name="pp", bufs=4, space="PSUM") as pp,
        ):
            # ---- persistent SBUF ----
            wqa = cpool.tile([128, CCH, 128], bf16, tag="wqa")
            wka = cpool.tile([128, CCH, 128], bf16, tag="wka")
            wqb = cpool.tile([128, CCH, 64], bf16, tag="wqb")
            wkb = cpool.tile([128, CCH, 64], bf16, tag="wkb")
            wv = cpool.tile([128, CCH, HD], bf16, tag="wv")
            wo = cpool.tile([64, HP, D], bf16, tag="wo")
            sh = cpool.tile([128, HP, 383], f32, tag="sh")
            msk = cpool.tile([128, KC], f32, tag="msk")
            cmc = cpool.tile([128, HP], f32, tag="cmc")
            cpc = cpool.tile([128, HP], f32, tag="cpc")
            # per-contraction-chunk tiles so each projection matmul only
            # waits on its own chunk's DMA, not the whole activation load
            qT = [cpool.tile([128, L], bf16, tag=f"qT{c}", name=f"qT{c}")
                  for c in range(CCH)]
            kvT = [cpool.tile([128, L], bf16, tag=f"kvT{c}", name=f"kvT{c}")
                   for c in range(CCH)]
            oacc = cpool.tile([128, 8, D], f32, tag="oacc")
            # heads 0,1 stacked on partitions 0-63 / 64-127
            QTa = cpool.tile([128, L], bf16, tag="QTa")
            KTa = cpool.tile([128, L], bf16, tag="KTa")
            # head 2: K on partitions 64-127 (straight from its col-tiled
            # projection), Q on 0-63 then DMA-duplicated to 64-127 so both
            # score operands live on the same partition half
            QTb = cpool.tile([128, L], bf16, tag="QTb")
            KTb = cpool.tile([128, L], bf16, tag="KTb")
            Vg = cpool.tile([128, KC, HP, 65], bf16, tag="Vg")
            AT = cpool.tile([64, HP, L], bf16, tag="AT")


            # single activation-table load covering both Exp and Ln; the
            # compile-time pass then sees every activation's table resident
            nc.scalar.add_instruction(mybir.InstLoadActFuncSet(
                name=nc.get_next_instruction_name(), ins=[], outs=[],
                act_func_set_id=nle_id))

            # ---- loads; inputs split across both HWDGE queues (kvT on
            # sync, qT on scalar behind the small weights) so the two
            # streams land in parallel and the projections start early ----
            nc.scalar.dma_start(out=wka[:], in_=wka_d.rearrange("(c p) n -> p c n", p=128))
            nc.scalar.dma_start(out=wqa[:], in_=wqa_d.rearrange("(c p) n -> p c n", p=128))
            nc.scalar.dma_start(out=wv[:], in_=wv_d.rearrange("(c p) n -> p c n", p=128))
            nc.scalar.dma_start(out=wqb[:], in_=wqb_d.rearrange("(c p) n -> p c n", p=128))
            nc.scalar.dma_start(out=wkb[:], in_=wkb_d.rearrange("(c p) n -> p c n", p=128))
            nc.scalar.dma_start(out=sh[:], in_=sh_d.rearrange("h p y -> p h y"))
            nc.scalar.dma_start(out=msk[:], in_=msk_d)
            nc.scalar.dma_start(out=cmc[:], in_=cm_d)
            nc.scalar.dma_start(out=cpc[:], in_=cp_d)
            nc.scalar.dma_start(out=wo[:], in_=wo_d)
            qT_r = qT_d.rearrange("(c p) n -> p c n", p=128)
            kvT_r = kvT_d.rearrange("(c p) n -> p c n", p=128)
            for c in range(CCH):
                nc.sync.dma_start(out=kvT[c][:], in_=kvT_r[:, c, :])
            for c in range(CCH):
                nc.scalar.dma_start(out=qT[c][:], in_=qT_r[:, c, :])

            # ---- Q/K projections (heads 0,1; head 2 is deferred into
            # q-half 0's attention phases as PE filler) ----
            for w_in, x_in, dst in ((wka, kvT, KTa), (wqa, qT, QTa)):
                for n in range(4):
                    nsl = slice(512 * n, 512 * n + 512)
                    ps = pp.tile([128, 512], f32, tag="pp", name=f"ps{dst.name}_{n}")
                    for c in range(CCH):
                        nc.tensor.matmul(
                            ps[:], lhsT=w_in[:, c, :], rhs=x_in[c][:, nsl],
                            start=(c == 0), stop=(c == CCH - 1),
                        )
                    nc.vector.tensor_copy(dst[:, nsl], ps[:])

            def qk2_proj(n):
                """head 2's Q and K projection n-chunk, col-tiled in one
                pass (Q on array cols 0-63, K on 64-127), separate PSUM
                banks so the accumulation groups' has_written clears stay
                apart."""
                nsl = slice(512 * n, 512 * n + 512)
                psq = pp.tile([128, 512], f32, tag="pp", name=f"psbq_{n}")
                psk = pp.tile([128, 512], f32, tag="pp", name=f"psbk_{n}")
                for c in range(CCH):
                    nc.tensor.matmul(
                        psq[0:64, :], lhsT=wqb[:, c, :], rhs=qT[c][:, nsl],
                        start=(c == 0), stop=(c == CCH - 1),
                    )
                    nc.tensor.matmul(
                        psk[64:128, :], lhsT=wkb[:, c, :], rhs=kvT[c][:, nsl],
                        start=(c == 0), stop=(c == CCH - 1),
                        tile_position=(0, 64),
                    )
                nc.vector.tensor_copy(QTb[0:64, nsl], psq[0:64, :])
                nc.vector.tensor_copy(KTb[64:128, nsl], psk[64:128, :])

            # mask column of V_aug, all key chunks at once
            mrep = bass.AP(msk[:].tensor, msk[:].offset,
                           [list(msk[:].ap[0]), [1, KC], [0, HP], [1, 1]])
            nc.vector.tensor_copy(Vg[:, :, :, 64:65], mrep)

            def v_proj(kc, h):
                """V projection chunk for one head -> Vg[:, kc, h, 0:64]
                (interleaved just-in-time into that head's q-half 0
                attention as PE filler work)."""
                ps_v = pp.tile([128, 512], f32, tag="pp", name=f"psv{kc}_{h}")
                for c in range(CCH):
                    nc.tensor.matmul(
                        ps_v[:, 0:64],
                        lhsT=kvT[c][:, 128 * kc:128 * kc + 128],
                        rhs=wv[:, c, 64 * h:64 * h + 64],
                        start=(c == 0), stop=(c == CCH - 1),
                    )
                nc.vector.tensor_copy(Vg[:, kc, h, 0:64], ps_v[:, 0:64])

            def band_add(s, h, kc, ha):
                """near-diagonal bias add (in place, PSUM); s covers
                columns [ha, ha+1024)."""
                qlo = max(0, 128 * kc - 128)
                qhi = min(L, 128 * kc + 255)
                x0 = (2047 + 128 * kc - qlo) - 1793
                a = max(qlo, ha)
                b = min(qhi, ha + 1024)
                if b > a:
                    sh_ap = sh[:, h, :]
                    rev = bass.AP(
                        sh_ap.tensor, sh_ap.offset + x0 - (a - qlo),
                        [list(sh_ap.ap[0]), [-1, b - a]],
                    )
                    nc.vector.tensor_add(
                        s[:, a - ha:b - ha], s[:, a - ha:b - ha], rev)

            def exp_split(s, h, kc, ha, name):
                """exp with bias cm everywhere (one ACT instruction); the
                k-q >= 128 region's true bias is cp, fixed up afterwards by
                a cheap DVE multiply with exp(cp-cm) (cpc carries it)."""
                p = ppool.tile([128, 1024], bf16, tag="p", name=name)
                wcp = max(0, 128 * kc - 128)
                wl = min(max(wcp - ha, 0), 1024)
                nc.scalar.activation(
                    p[:], s[:], Exp, bias=cmc[:, h:h + 1], scale=1.0)
                if wl > 0:
                    nc.vector.tensor_scalar_mul(
                        p[:, 0:wl], p[:, 0:wl], cpc[:, h:h + 1])
                return p

            def normalize(pvs, h, qh):
                """pvs: two [65, 512] PSUM accumulators (numerator rows
                0-63, denominator row 64) -> AT[:, h, qh*1024 : +1024]."""
                pvsb = npool.tile([65, 1024], bf16, tag="pvsb",
                                  name=f"pvsb{qh}_{h}")
                for j in range(2):
                    nc.vector.tensor_copy(pvsb[:, 512 * j:512 * j + 512],
                                          pvs[j][:])
                lns = npool.tile([1, 1024], f32, tag="lns", name=f"l{qh}_{h}")
                nc.scalar.activation(lns[:], pvsb[64:65, :], Ln)
                inv = npool.tile([1, 1024], bf16, tag="inv", name=f"i{qh}_{h}")
                nc.scalar.activation(inv[:], lns[:], Exp, scale=-1.0)
                invb = npool.tile([64, 1024], bf16, tag="invb",
                                  name=f"ib{qh}_{h}")
                nc.gpsimd.partition_broadcast(invb[:], inv[:])
                qsl = slice(1024 * qh, 1024 * qh + 1024)
                nc.vector.tensor_mul(AT[:, h, qsl], pvsb[0:64, :], invb[:])

            _ostate = {}

            def out_proj_unit(qc, nlo):
                """one output-projection unit: 3 accumulating matmuls for
                query rows [128*qc, +128), output cols [nlo, nlo+nw); the
                second unit of a row chunk completes the tile and DMAs it."""
                nw = 512 if nlo == 0 else 256
                if nlo == 0:
                    _ostate[qc] = opool.tile([128, D], bf16, tag="o",
                                             name=f"o{qc}")
                o = _ostate[qc]
                ps_o = pp.tile([128, 512], f32, tag="pp", name=f"po{qc}_{nlo}")
                for h in range(HP):
                    nc.tensor.matmul(
                        ps_o[:, 0:nw],
                        lhsT=AT[:, h, 128 * qc:128 * qc + 128],
                        rhs=wo[:, h, nlo:nlo + nw],
                        start=(h == 0), stop=(h == HP - 1),
                    )
                nc.vector.tensor_copy(o[:, nlo:nlo + nw], ps_o[:, 0:nw])
                if nlo != 0:
                    nc.sync.dma_start(
                        out=out_d[128 * qc:128 * qc + 128, :], in_=o[:])

            def out_proj_a(qc, nlo, heads):
                """q-half-1 output projection, stage A: partial sum over
                `heads` into the SBUF accumulator (interleaved as PE filler
                before the last head's attention finishes)."""
                nw = 512 if nlo == 0 else 256
                ps_o = pp.tile([128, 512], f32, tag="pp",
                               name=f"pa{qc}_{nlo}")
                for i, h in enumerate(heads):
                    nc.tensor.matmul(
                        ps_o[:, 0:nw],
                        lhsT=AT[:, h, 128 * qc:128 * qc + 128],
                        rhs=wo[:, h, nlo:nlo + nw],
                        start=(i == 0), stop=(i == len(heads) - 1),
                    )
                nc.vector.tensor_copy(oacc[:, qc - 8, nlo:nlo + nw],
                                      ps_o[:, 0:nw])

            def out_proj_b(qc, nlo, h):
                """stage B: last head's contribution + accumulator -> out."""
                nw = 512 if nlo == 0 else 256
                if nlo == 0:
                    _ostate[qc] = opool.tile([128, D], bf16, tag="o",
                                             name=f"o{qc}")
                o = _ostate[qc]
                ps_o = pp.tile([128, 512], f32, tag="pp",
                               name=f"pb{qc}_{nlo}")
                nc.tensor.matmul(ps_o[:, 0:nw],
                                 lhsT=AT[:, h, 128 * qc:128 * qc + 128],
                                 rhs=wo[:, h, nlo:nlo + nw],
                                 start=True, stop=True)
                nc.vector.tensor_add(o[:, nlo:nlo + nw],
                                     oacc[:, qc - 8, nlo:nlo + nw],
                                     ps_o[:, 0:nw])
                if nlo != 0:
                    nc.sync.dma_start(
                        out=out_d[128 * qc:128 * qc + 128, :], in_=o[:])

            _hb = [0]

            def heartbeat(tile=None):
                """dummy matmul into a scratch PSUM slot; its only purpose
                is keeping the PE's HAM activity monitor from re-throttling
                the clock to 1.2 GHz during ACT-bound stretches. Passing a
                shared tile serializes consecutive heartbeats (write-after-
                write), spreading them across a known stall instead of
                letting the scheduler fire them all at once."""
                _hb[0] += 1
                hb = tile if tile is not None else pp.tile(
                    [128, 512], f32, tag="pp", name=f"hb{_hb[0]}")
                nc.tensor.matmul(hb[:], lhsT=wqa[:, 0, :],
                                 rhs=KTa[:, 0:512], start=True, stop=True)

            def head_ops(h):
                if h == 0:
                    return (lambda kc: KTa[0:64, 128 * kc:128 * kc + 128],
                            lambda lo: QTa[0:64, lo:lo + 512])
                if h == 1:
                    return (lambda kc: KTa[64:128, 128 * kc:128 * kc + 128],
                            lambda lo: QTa[64:128, lo:lo + 512])
                return (lambda kc: KTb[64:128, 128 * kc:128 * kc + 128],
                        lambda lo: QTb[64:128, lo:lo + 512])

            # ---- fused attention, transposed orientation, q-half major ----
            # PE filler schedule keeps the tensor engine's queue from
            # draining (and its HAM clock gate from re-throttling):
            #   qh0/h: that head's V-projection chunks just-in-time, plus
            #          head 2's deferred Q/K projection split over h0/h1
            #   qh1 runs heads (2, 0, 1): q-half 0's output projection
            #          fills h2 and h0; q-half 1's own output projection
            #          runs 2-stage, heads {2,0} filling the h1 phase and
            #          only head 1's single matmul per unit in the tail
            qh0_units = iter([(qc, nlo) for qc in range(8)
                              for nlo in (0, 512)])
            qh1a_units = iter([(qc, nlo) for qc in range(8, 16)
                               for nlo in (0, 512)])
            for qh in range(2):
                ha = 1024 * qh
                heads = (0, 1, 2) if qh == 0 else (2, 0, 1)
                for hi, h in enumerate(heads):
                    kslice, qslice = head_ops(h)
                    pvs = [pp.tile([65, 512], f32, tag="pp",
                                   name=f"pv{qh}_{h}_{j}") for j in range(2)]
                    if qh == 0:
                        v_proj(0, h)
                        v_proj(1, h)
                    for kc in range(KC):
                        s = sp.tile([128, 1024], f32, tag="sp",
                                    name=f"s{qh}_{h}_{kc}")
                        for jj in range(2):
                            nc.tensor.matmul(
                                s[:, 512 * jj:512 * jj + 512],
                                lhsT=kslice(kc), rhs=qslice(ha + 512 * jj),
                                start=True, stop=True)
                        band_add(s, h, kc, ha)
                        p = exp_split(s, h, kc, ha, f"p{qh}_{h}_{kc}")
                        for jj in range(2):
                            nc.tensor.matmul(
                                pvs[jj][:],
                                lhsT=Vg[:, kc, h, :],
                                rhs=p[:, 512 * jj:512 * jj + 512],
                                start=(kc == 0), stop=(kc == KC - 1))
                        if qh == 0 and kc + 2 < KC:
                            v_proj(kc + 2, h)
                        if qh == 0 and hi < 2 and kc % 8 == 4:
                            qk2_proj(2 * hi + kc // 8)
                        if qh == 1 and hi < 2:
                            if kc % 2 == 0:
                                unit = next(qh0_units, None)
                                if unit is not None:
                                    out_proj_unit(*unit)
                            else:
                                heartbeat()
                        if qh == 1 and hi == 2:
                            unit = next(qh1a_units, None)
                            if unit is not None:
                                out_proj_a(*unit, heads=(2, 0))
                    normalize(pvs, h, qh)
                    if qh == 0 and h == 1:
                        # head-2 scores contract on partitions 64-127; its
                        # Q was projected onto 0-63 during the h0/h1 phases
                        nc.sync.dma_start(out=QTb[64:128, :], in_=QTb[0:64, :])

            # ---- tail: last head's output-projection contribution ----
            # (heartbeats bridge the final normalize's latency so the tail
            # matmuls run at full clock)
            for _ in range(6):
                heartbeat()
            for qc in range(8, L // 128):
                out_proj_b(qc, 0, 1)
                out_proj_b(qc, 512, 1)

    nc.compile()
    return nc


def _get_program():
    with _lock:
        if "nc" not in _cache:
            _cache["nc"] = _build_program()
        return _cache["nc"]


def _host_prep(core, query, key_value, key_padding_mask, Wq, Wk, Wv, Wo, rel_emb):
    import ml_dtypes

    bf16 = ml_dtypes.bfloat16
    b, g = core // 4, core % 4
    mask = key_padding_mask[b].astype(np.float32)
    kv = key_value[b] * mask[:, None]
    qT = np.ascontiguousarray(query[b].T).astype(bf16)
    kvT = np.ascontiguousarray(kv.T).astype(bf16)
    sl = slice(HD * g, HD * (g + 1))
    wq = np.ascontiguousarray(Wq[:, sl])
    wk = np.ascontiguousarray(Wk[:, sl]) * np.float32(DK ** -0.5)
    wv = np.ascontiguousarray(Wv[:, sl]).astype(bf16)
    wo = np.ascontiguousarray(
        Wo[sl].reshape(HP, 64, D).transpose(1, 0, 2)).astype(bf16)

    d = np.arange(-2047, 2048)
    buckets = _np_bucket(d)
    heads = [HP * g + i for i in range(HP)]
    t = rel_emb[buckets][:, heads].astype(np.float32)  # [4095, HP]
    cm = t[0]
    cp = t[-1]
    # sh[h, p, y] = t[y + 1793 + p, h] - cm[h]
    p_i = np.arange(128)[:, None]
    y_i = np.arange(383)[None, :]
    sh = np.ascontiguousarray(
        (t[y_i + 1793 + p_i] - cm[None, None, :]).transpose(2, 0, 1))
    msk = np.ascontiguousarray(mask.reshape(KC, 128).T)
    cmc = np.ascontiguousarray(np.broadcast_to(cm[None, :], (128, HP)))
    # the kernel exps everything with bias cm and multiplies the k-q >= 128
    # region by exp(cp - cm) afterwards
    cpc = np.ascontiguousarray(
        np.broadcast_to(np.exp(cp - cm)[None, :], (128, HP)))
    return {
        "qT": qT, "kvT": kvT,
        "wqa": np.ascontiguousarray(wq[:, 0:128]).astype(bf16),
        "wka": np.ascontiguousarray(wk[:, 0:128]).astype(bf16),
        "wqb": np.ascontiguousarray(wq[:, 128:192]).astype(bf16),
        "wkb": np.ascontiguousarray(wk[:, 128:192]).astype(bf16),
        "wv": wv, "wo": wo,
        "sh": sh.astype(np.float32), "msk": msk,
        "cm": cmc.astype(np.float32), "cp": cpc.astype(np.float32),
    }


def make_in_maps(**inputs):
    return [_host_prep(c, **inputs) for c in range(NCORES)]


def kernel(query, key_value, key_padding_mask, Wq, Wk, Wv, Wo, rel_emb,
           _results_hook=None, _run_kwargs=None):
    from concourse.bass_utils import run_bass_kernel_spmd

    inputs = dict(query=np.asarray(query), key_value=np.asarray(key_value),
                  key_padding_mask=np.asarray(key_padding_mask),
                  Wq=np.asarray(Wq, np.float32), Wk=np.asarray(Wk, np.float32),
                  Wv=np.asarray(Wv, np.float32), Wo=np.asarray(Wo, np.float32),
                  rel_emb=np.asarray(rel_emb, np.float32))
    nc = _get_program()
    in_maps = make_in_maps(**inputs)
    res = run_bass_kernel_spmd(nc, in_maps, core_ids=list(range(NCORES)),
                               **(_run_kwargs or {}))
    if _results_hook is not None:
        _results_hook(res)
    out = np.zeros((B, L, D), np.float32)
    for c in range(NCORES):
        out[c // 4] += res.results[c]["out_p"].astype(np.float32)
    return out


# revision 37
# speedup vs baseline: 1.1813x; 1.0254x over previous
"""Trainium2 Bass kernel for nn_MultiHeadAttention_44908178047033.

T5-style MHA (relative-position bias, bidirectional) over
B=2, L=2048, D=768, H=12, DK=64.

Sharding: 8 cores = 2 batches x 4 head-groups (3 heads each).
Each core computes Q/K/V projections for its (batch, 3 heads), fused
transposed-orientation attention (scores kept as S^T [k, q] so the
softmax denominator and the PV contraction both run as PE matmuls
without transposing the probability matrix), and a partial output
projection. Host sums the 4 per-head-group partials per batch.

v3 perf structure (from trace analysis of the f32r baseline and v2):
- everything bf16: halves input DMA, runs all matmuls at bf16 rate
- single ACT table preload (natural_log_exp_and_others) so the Ln/Exp
  softmax normalization never swaps activation tables mid-kernel
- q-half-major loop with sequential heads: only 2 PV accumulator banks
  and 2 double-buffered score tiles are live, leaving 2 PSUM banks for
  interleaved filler matmuls
- the PE HAM clock gate re-throttles to 1.2 GHz after any ~3.4us idle
  window and only re-warms after ~3.4us of continuous work, so the V
  projection is interleaved into q-half 0's attention and the output
  projection of q-half 0 into q-half 1's attention: the PE instruction
  queue never drains at phase transitions and stays at 2.4 GHz

Relative-position bias: the T5 bias f(k-q) is constant for |k-q| >= 128
(log-bucketing saturates), so
  exp(s + f) = exp(s + cm)            for k-q <= -128  (ACT bias, free)
             = exp(s + cp)            for k-q >= +128  (ACT bias, free)
             = exp(s + cm + (f - cm)) for |k-q| < 128  (DVE add from a
               host-precomputed per-partition shifted Toeplitz table,
               read with a negative free-dim stride)
"""

import math
import sys
import threading

import numpy as np

sys.path.insert(0, "/opt/trn_rl_repo")

B, L, D = 2, 2048, 768
H, DK = 12, 64
NUM_BUCKETS, MAX_DIST = 32, 128
HP = 3            # heads per core
HD = HP * DK      # 192 cols per head-group
NCORES = 8
KC = 16           # key chunks of 128
CCH = 6           # contraction chunks of 128 over D

_cache = {}
_lock = threading.Lock()


def _np_bucket(d):
    rel = d
    ret = np.zeros_like(rel)
    n = -rel
    nb = NUM_BUCKETS // 2
    ret = ret + (n < 0).astype(np.int32) * nb
    n = np.abs(n)
    mx = nb // 2
    is_small = n < mx
    n_safe = np.maximum(n, 1).astype(np.float32)
    vl = mx + (
        np.log(n_safe / mx) / math.log(MAX_DIST / mx) * (nb - mx)
    ).astype(np.int32)
    vl = np.minimum(vl, nb - 1)
    return ret + np.where(is_small, n, vl)


def _build_program():
    import concourse.bacc as bacc
    import concourse.bass as bass
    import concourse.mybir as mybir
    import concourse.tile as tile
    from concourse.hw_specs import get_activation_tables

    dt = mybir.dt
    f32, bf16 = dt.float32, dt.bfloat16
    Exp, Ln = mybir.ActivationFunctionType.Exp, mybir.ActivationFunctionType.Ln

    act_sets = list(get_activation_tables("gen3").keys())
    nle_id = act_sets.index("natural_log_exp_and_others")

    nc = bacc.Bacc("TRN2", target_bir_lowering=False, debug=False,
                   num_devices=NCORES)

    qT_d = nc.dram_tensor("qT", [D, L], bf16, kind="ExternalInput").ap()
    kvT_d = nc.dram_tensor("kvT", [D, L], bf16, kind="ExternalInput").ap()
    wqa_d = nc.dram_tensor("wqa", [D, 128], bf16, kind="ExternalInput").ap()
    wka_d = nc.dram_tensor("wka", [D, 128], bf16, kind="ExternalInput").ap()
    wqb_d = nc.dram_tensor("wqb", [D, 64], bf16, kind="ExternalInput").ap()
    wkb_d = nc.dram_tensor("wkb", [D, 64], bf16, kind="ExternalInput").ap()
    wv_d = nc.dram_tensor("wv", [D, HD], bf16, kind="ExternalInput").ap()
    wo_d = nc.dram_tensor("wo", [64, HP, D], bf16, kind="ExternalInput").ap()
    sh_d = nc.dram_tensor("sh", [HP, 128, 383], f32, kind="ExternalInput").ap()
    msk_d = nc.dram_tensor("msk", [128, KC], f32, kind="ExternalInput").ap()
    cm_d = nc.dram_tensor("cm", [128, HP], f32, kind="ExternalInput").ap()
    cp_d = nc.dram_tensor("cp", [128, HP], f32, kind="ExternalInput").ap()
    out_d = nc.dram_tensor("out_p", [L, D], bf16, kind="ExternalOutput").ap()

    with tile.TileContext(nc) as tc:
        with (
            tc.tile_pool(name="const", bufs=1) as cpool,
            tc.tile_pool(name="p", bufs=4) as ppool,
            tc.tile_pool(name="o", bufs=2) as opool,
            tc.tile_pool(name="nrm", bufs=4) as npool,
            tc.tile_pool(name="sp", bufs=2, space="PSUM") as sp,
            tc.tile_pool(name="pp", bufs=4, space="PSUM") as pp,
        ):
            # ---- persistent SBUF ----
            wqa = cpool.tile([128, CCH, 128], bf16, tag="wqa")
            wka = cpool.tile([128, CCH, 128], bf16, tag="wka")
            wqb = cpool.tile([128, CCH, 64], bf16, tag="wqb")
            wkb = cpool.tile([128, CCH, 64], bf16, tag="wkb")
            wv = cpool.tile([128, CCH, HD], bf16, tag="wv")
            wo = cpool.tile([64, HP, D], bf16, tag="wo")
            sh = cpool.tile([128, HP, 383], f32, tag="sh")
            msk = cpool.tile([128, KC], f32, tag="msk")
            cmc = cpool.tile([128, HP], f32, tag="cmc")
            cpc = cpool.tile([128, HP], f32, tag="cpc")
            # per-contraction-chunk tiles so each projection matmul only
            # waits on its own chunk's DMA, not the whole activation load
            qT = [cpool.tile([128, L], bf16, tag=f"qT{c}", name=f"qT{c}")
                  for c in range(CCH)]
            kvT = [cpool.tile([128, L], bf16, tag=f"kvT{c}", name=f"kvT{c}")
                   for c in range(CCH)]
            oacc = cpool.tile([128, 8, D], f32, tag="oacc")
            # heads 0,1 stacked on partitions 0-63 / 64-127
            QTa = cpool.tile([128, L], bf16, tag="QTa")
            KTa = cpool.tile([128, L], bf16, tag="KTa")
            # head 2: K on partitions 64-127 (straight from its col-tiled
            # projection), Q on 0-63 then DMA-duplicated to 64-127 so both
            # score operands live on the same partition half
            QTb = cpool.tile([128, L], bf16, tag="QTb")
            KTb = cpool.tile([128, L], bf16, tag="KTb")
            Vg = cpool.tile([128, KC, HP, 65], bf16, tag="Vg")
            AT = cpool.tile([64, HP, L], bf16, tag="AT")


            # single activation-table load covering both Exp and Ln; the
            # compile-time pass then sees every activation's table resident
            nc.scalar.add_instruction(mybir.InstLoadActFuncSet(
                name=nc.get_next_instruction_name(), ins=[], outs=[],
                act_func_set_id=nle_id))

            # ---- loads; inputs split across both HWDGE queues (kvT on
            # sync, qT on scalar behind the small weights) so the two
            # streams land in parallel and the projections start early ----
            nc.scalar.dma_start(out=wka[:], in_=wka_d.rearrange("(c p) n -> p c n", p=128))
            nc.scalar.dma_start(out=wqa[:], in_=wqa_d.rearrange("(c p) n -> p c n", p=128))
            nc.scalar.dma_start(out=wv[:], in_=wv_d.rearrange("(c p) n -> p c n", p=128))
            nc.scalar.dma_start(out=wqb[:], in_=wqb_d.rearrange("(c p) n -> p c n", p=128))
            nc.scalar.dma_start(out=wkb[:], in_=wkb_d.rearrange("(c p) n -> p c n", p=128))
            nc.scalar.dma_start(out=sh[:], in_=sh_d.rearrange("h p y -> p h y"))
            nc.scalar.dma_start(out=msk[:], in_=msk_d)
            nc.scalar.dma_start(out=cmc[:], in_=cm_d)
            nc.scalar.dma_start(out=cpc[:], in_=cp_d)
            nc.scalar.dma_start(out=wo[:], in_=wo_d)
            qT_r = qT_d.rearrange("(c p) n -> p c n", p=128)
            kvT_r = kvT_d.rearrange("(c p) n -> p c n", p=128)
            for c in range(CCH):
                nc.sync.dma_start(out=kvT[c][:], in_=kvT_r[:, c, :])
            for c in range(CCH):
                nc.scalar.dma_start(out=qT[c][:], in_=qT_r[:, c, :])

            # ---- Q/K projections (heads 0,1; head 2 is deferred into
            # q-half 0's attention phases as PE filler) ----
            for w_in, x_in, dst in ((wka, kvT, KTa), (wqa, qT, QTa)):
                for n in range(4):
                    nsl = slice(512 * n, 512 * n + 512)
                    ps = pp.tile([128, 512], f32, tag="pp", name=f"ps{dst.name}_{n}")
                    for c in range(CCH):
                        nc.tensor.matmul(
                            ps[:], lhsT=w_in[:, c, :], rhs=x_in[c][:, nsl],
                            start=(c == 0), stop=(c == CCH - 1),
                        )
                    nc.vector.tensor_copy(dst[:, nsl], ps[:])

            def qk2_proj(n):
                """head 2's Q and K projection n-chunk, col-tiled in one
                pass (Q on array cols 0-63, K on 64-127), separate PSUM
                banks so the accumulation groups' has_written clears stay
                apart."""
                nsl = slice(512 * n, 512 * n + 512)
                psq = pp.tile([128, 512], f32, tag="pp", name=f"psbq_{n}")
                psk = pp.tile([128, 512], f32, tag="pp", name=f"psbk_{n}")
                for c in range(CCH):
                    nc.tensor.matmul(
                        psq[0:64, :], lhsT=wqb[:, c, :], rhs=qT[c][:, nsl],
                        start=(c == 0), stop=(c == CCH - 1),
                    )
                    nc.tensor.matmul(
                        psk[64:128, :], lhsT=wkb[:, c, :], rhs=kvT[c][:, nsl],
                        start=(c == 0), stop=(c == CCH - 1),
                        tile_position=(0, 64),
                    )
                nc.vector.tensor_copy(QTb[0:64, nsl], psq[0:64, :])
                nc.vector.tensor_copy(KTb[64:128, nsl], psk[64:128, :])

            # mask column of V_aug, all key chunks at once
            mrep = bass.AP(msk[:].tensor, msk[:].offset,
                           [list(msk[:].ap[0]), [1, KC], [0, HP], [1, 1]])
            nc.vector.tensor_copy(Vg[:, :, :, 64:65], mrep)

            def v_proj(kc, h):
                """V projection chunk for one head -> Vg[:, kc, h, 0:64]
                (interleaved just-in-time into that head's q-half 0
                attention as PE filler work)."""
                ps_v = pp.tile([128, 512], f32, tag="pp", name=f"psv{kc}_{h}")
                for c in range(CCH):
                    nc.tensor.matmul(
                        ps_v[:, 0:64],
                        lhsT=kvT[c][:, 128 * kc:128 * kc + 128],
                        rhs=wv[:, c, 64 * h:64 * h + 64],
                        start=(c == 0), stop=(c == CCH - 1),
                    )
                nc.vector.tensor_copy(Vg[:, kc, h, 0:64], ps_v[:, 0:64])

            def band_add(s, h, kc, ha):
                """near-diagonal bias add (in place, PSUM); s covers
                columns [ha, ha+1024)."""
                qlo = max(0, 128 * kc - 128)
                qhi = min(L, 128 * kc + 255)
                x0 = (2047 + 128 * kc - qlo) - 1793
                a = max(qlo, ha)
                b = min(qhi, ha + 1024)
                if b > a:
                    sh_ap = sh[:, h, :]
                    rev = bass.AP(
                        sh_ap.tensor, sh_ap.offset + x0 - (a - qlo),
                        [list(sh_ap.ap[0]), [-1, b - a]],
                    )
                    nc.vector.tensor_add(
                        s[:, a - ha:b - ha], s[:, a - ha:b - ha], rev)

            def exp_split(s, h, kc, ha, name):
                """exp with bias cm everywhere (one ACT instruction); the
                k-q >= 128 region's true bias is cp, fixed up afterwards by
                a cheap DVE multiply with exp(cp-cm) (cpc carries it)."""
                p = ppool.tile([128, 1024], bf16, tag="p", name=name)
                wcp = max(0, 128 * kc - 128)
                wl = min(max(wcp - ha, 0), 1024)
                nc.scalar.activation(
                    p[:], s[:], Exp, bias=cmc[:, h:h + 1], scale=1.0)
                if wl > 0:
                    nc.vector.tensor_scalar_mul(
                        p[:, 0:wl], p[:, 0:wl], cpc[:, h:h + 1])
                return p

            def normalize(pvs, h, qh):
                """pvs: two [65, 512] PSUM accumulators (numerator rows
                0-63, denominator row 64) -> AT[:, h, qh*1024 : +1024]."""
                pvsb = npool.tile([65, 1024], bf16, tag="pvsb",
                                  name=f"pvsb{qh}_{h}")
                for j in range(2):
                    nc.vector.tensor_copy(pvsb[:, 512 * j:512 * j + 512],
                                          pvs[j][:])
                lns = npool.tile([1, 1024], f32, tag="lns", name=f"l{qh}_{h}")
                nc.scalar.activation(lns[:], pvsb[64:65, :], Ln)
                inv = npool.tile([1, 1024], bf16, tag="inv", name=f"i{qh}_{h}")
                nc.scalar.activation(inv[:], lns[:], Exp, scale=-1.0)
                invb = npool.tile([64, 1024], bf16, tag="invb",
                                  name=f"ib{qh}_{h}")
                nc.gpsimd.partition_broadcast(invb[:], inv[:])
                qsl = slice(1024 * qh, 1024 * qh + 1024)
                nc.vector.tensor_mul(AT[:, h, qsl], pvsb[0:64, :], invb[:])

            _ostate = {}

            def out_proj_unit(qc, nlo):
                """one output-projection unit: 3 accumulating matmuls for
                query rows [128*qc, +128), output cols [nlo, nlo+nw); the
                second unit of a row chunk completes the tile and DMAs it."""
                nw = 512 if nlo == 0 else 256
                if nlo == 0:
                    _ostate[qc] = opool.tile([128, D], bf16, tag="o",
                                             name=f"o{qc}")
                o = _ostate[qc]
                ps_o = pp.tile([128, 512], f32, tag="pp", name=f"po{qc}_{nlo}")
                for h in range(HP):
                    nc.tensor.matmul(
                        ps_o[:, 0:nw],
                        lhsT=AT[:, h, 128 * qc:128 * qc + 128],
                        rhs=wo[:, h, nlo:nlo + nw],
                        start=(h == 0), stop=(h == HP - 1),
                    )
                nc.vector.tensor_copy(o[:, nlo:nlo + nw], ps_o[:, 0:nw])
                if nlo != 0:
                    nc.sync.dma_start(
                        out=out_d[128 * qc:128 * qc + 128, :], in_=o[:])

            def out_proj_a(qc, nlo, heads):
                """q-half-1 output projection, stage A: partial sum over
                `heads` into the SBUF accumulator (interleaved as PE filler
                before the last head's attention finishes)."""
                nw = 512 if nlo == 0 else 256
                ps_o = pp.tile([128, 512], f32, tag="pp",
                               name=f"pa{qc}_{nlo}")
                for i, h in enumerate(heads):
                    nc.tensor.matmul(
                        ps_o[:, 0:nw],
                        lhsT=AT[:, h, 128 * qc:128 * qc + 128],
                        rhs=wo[:, h, nlo:nlo + nw],
                        start=(i == 0), stop=(i == len(heads) - 1),
                    )
                nc.vector.tensor_copy(oacc[:, qc - 8, nlo:nlo + nw],
                                      ps_o[:, 0:nw])

            def out_proj_b(qc, nlo, h):
                """stage B: last head's contribution + accumulator -> out."""
                nw = 512 if nlo == 0 else 256
                if nlo == 0:
                    _ostate[qc] = opool.tile([128, D], bf16, tag="o",
                                             name=f"o{qc}")
                o = _ostate[qc]
                ps_o = pp.tile([128, 512], f32, tag="pp",
                               name=f"pb{qc}_{nlo}")
                nc.tensor.matmul(ps_o[:, 0:nw],
                                 lhsT=AT[:, h, 128 * qc:128 * qc + 128],
                                 rhs=wo[:, h, nlo:nlo + nw],
                                 start=True, stop=True)
                nc.vector.tensor_add(o[:, nlo:nlo + nw],
                                     oacc[:, qc - 8, nlo:nlo + nw],
                                     ps_o[:, 0:nw])
                if nlo != 0:
                    nc.sync.dma_start(
                        out=out_d[128 * qc:128 * qc + 128, :], in_=o[:])

            _hb = [0]

            def heartbeat(tile=None):
                """dummy matmul into a scratch PSUM slot; its only purpose
                is keeping the PE's HAM activity monitor from re-throttling
                the clock to 1.2 GHz during ACT-bound stretches. Passing a
                shared tile serializes consecutive heartbeats (write-after-
                write), spreading them across a known stall instead of
                letting the scheduler fire them all at once."""
                _hb[0] += 1
                hb = tile if tile is not None else pp.tile(
                    [128, 512], f32, tag="pp", name=f"hb{_hb[0]}")
                nc.tensor.matmul(hb[:], lhsT=wqa[:, 0, :],
                                 rhs=KTa[:, 0:512], start=True, stop=True)

            def head_ops(h):
                if h == 0:
                    return (lambda kc: KTa[0:64, 128 * kc:128 * kc + 128],
                            lambda lo: QTa[0:64, lo:lo + 512])
                if h == 1:
                    return (lambda kc: KTa[64:128, 128 * kc:128 * kc + 128],
                            lambda lo: QTa[64:128, lo:lo + 512])
                return (lambda kc: KTb[64:128, 128 * kc:128 * kc + 128],
                        lambda lo: QTb[64:128, lo:lo + 512])

            # ---- fused attention, transposed orientation, q-half major ----
            # PE filler schedule keeps the tensor engine's queue from
            # draining (and its HAM clock gate from re-throttling):
            #   qh0/h: that head's V-projection chunks just-in-time, plus
            #          head 2's deferred Q/K projection split over h0/h1
            #   qh1 runs heads (2, 0, 1): q-half 0's output projection
            #          fills h2 and h0; q-half 1's own output projection
            #          runs 2-stage, heads {2,0} filling the h1 phase and
            #          only head 1's single matmul per unit in the tail
            qh0_units = iter([(qc, nlo) for qc in range(8)
                              for nlo in (0, 512)])
            qh1a_units = iter([(qc, nlo) for qc in range(8, 16)
                               for nlo in (0, 512)])
            for qh in range(2):
                ha = 1024 * qh
                heads = (0, 1, 2) if qh == 0 else (2, 0, 1)
                for hi, h in enumerate(heads):
                    kslice, qslice = head_ops(h)
                    pvs = [pp.tile([65, 512], f32, tag="pp",
                                   name=f"pv{qh}_{h}_{j}") for j in range(2)]
                    if qh == 0:
                        v_proj(0, h)
                        v_proj(1, h)
                    for kc in range(KC):
                        s = sp.tile([128, 1024], f32, tag="sp",
                                    name=f"s{qh}_{h}_{kc}")
                        for jj in range(2):
                            nc.tensor.matmul(
                                s[:, 512 * jj:512 * jj + 512],
                                lhsT=kslice(kc), rhs=qslice(ha + 512 * jj),
                                start=True, stop=True)
                        band_add(s, h, kc, ha)
                        p = exp_split(s, h, kc, ha, f"p{qh}_{h}_{kc}")
                        for jj in range(2):
                            nc.tensor.matmul(
                                pvs[jj][:],
                                lhsT=Vg[:, kc, h, :],
                                rhs=p[:, 512 * jj:512 * jj + 512],
                                start=(kc == 0), stop=(kc == KC - 1))
                        if qh == 0 and kc + 2 < KC:
                            v_proj(kc + 2, h)
                        if qh == 0 and hi < 2 and kc % 8 == 4:
                            qk2_proj(2 * hi + kc // 8)
                        if qh == 1 and hi < 2:
                            if kc % 2 == 0:
                                unit = next(qh0_units, None)
                                if unit is not None:
                                    out_proj_unit(*unit)
                            else:
                                heartbeat()
                        if qh == 1 and hi == 2:
                            unit = next(qh1a_units, None)
                            if unit is not None:
                                out_proj_a(*unit, heads=(2, 0))
                    normalize(pvs, h, qh)
                    if qh == 0 and h == 1:
                        # head-2 scores contract on partitions 64-127; its
                        # Q was projected onto 0-63 during the h0/h1 phases
                        nc.sync.dma_start(out=QTb[64:128, :], in_=QTb[0:64, :])

            # ---- tail: last head's output-projection contribution ----
            # (a write-after-write-serialized heartbeat chain spreads the
            # dummies across the final normalize's latency so the tail
            # matmuls run at full clock)
            hbt = pp.tile([128, 512], f32, tag="pp", name="hbt")
            for _ in range(8):
                heartbeat(hbt)
            for qc in range(8, L // 128):
                out_proj_b(qc, 0, 1)
                out_proj_b(qc, 512, 1)

    nc.compile()
    return nc


def _get_program():
    with _lock:
        if "nc" not in _cache:
            _cache["nc"] = _build_program()
        return _cache["nc"]


def _host_prep(core, query, key_value, key_padding_mask, Wq, Wk, Wv, Wo, rel_emb):
    import ml_dtypes

    bf16 = ml_dtypes.bfloat16
    b, g = core // 4, core % 4
    mask = key_padding_mask[b].astype(np.float32)
    kv = key_value[b] * mask[:, None]
    qT = np.ascontiguousarray(query[b].T).astype(bf16)
    kvT = np.ascontiguousarray(kv.T).astype(bf16)
    sl = slice(HD * g, HD * (g + 1))
    wq = np.ascontiguousarray(Wq[:, sl])
    wk = np.ascontiguousarray(Wk[:, sl]) * np.float32(DK ** -0.5)
    wv = np.ascontiguousarray(Wv[:, sl]).astype(bf16)
    wo = np.ascontiguousarray(
        Wo[sl].reshape(HP, 64, D).transpose(1, 0, 2)).astype(bf16)

    d = np.arange(-2047, 2048)
    buckets = _np_bucket(d)
    heads = [HP * g + i for i in range(HP)]
    t = rel_emb[buckets][:, heads].astype(np.float32)  # [4095, HP]
    cm = t[0]
    cp = t[-1]
    # sh[h, p, y] = t[y + 1793 + p, h] - cm[h]
    p_i = np.arange(128)[:, None]
    y_i = np.arange(383)[None, :]
    sh = np.ascontiguousarray(
        (t[y_i + 1793 + p_i] - cm[None, None, :]).transpose(2, 0, 1))
    msk = np.ascontiguousarray(mask.reshape(KC, 128).T)
    cmc = np.ascontiguousarray(np.broadcast_to(cm[None, :], (128, HP)))
    # the kernel exps everything with bias cm and multiplies the k-q >= 128
    # region by exp(cp - cm) afterwards
    cpc = np.ascontiguousarray(
        np.broadcast_to(np.exp(cp - cm)[None, :], (128, HP)))
    return {
        "qT": qT, "kvT": kvT,
        "wqa": np.ascontiguousarray(wq[:, 0:128]).astype(bf16),
        "wka": np.ascontiguousarray(wk[:, 0:128]).astype(bf16),
        "wqb": np.ascontiguousarray(wq[:, 128:192]).astype(bf16),
        "wkb": np.ascontiguousarray(wk[:, 128:192]).astype(bf16),
        "wv": wv, "wo": wo,
        "sh": sh.astype(np.float32), "msk": msk,
        "cm": cmc.astype(np.float32), "cp": cpc.astype(np.float32),
    }


def make_in_maps(**inputs):
    return [_host_prep(c, **inputs) for c in range(NCORES)]


def kernel(query, key_value, key_padding_mask, Wq, Wk, Wv, Wo, rel_emb,
           _results_hook=None, _run_kwargs=None):
    from concourse.bass_utils import run_bass_kernel_spmd

    inputs = dict(query=np.asarray(query), key_value=np.asarray(key_value),
                  key_padding_mask=np.asarray(key_padding_mask),
                  Wq=np.asarray(Wq, np.float32), Wk=np.asarray(Wk, np.float32),
                  Wv=np.asarray(Wv, np.float32), Wo=np.asarray(Wo, np.float32),
                  rel_emb=np.asarray(rel_emb, np.float32))
    nc = _get_program()
    in_maps = make_in_maps(**inputs)
    res = run_bass_kernel_spmd(nc, in_maps, core_ids=list(range(NCORES)),
                               **(_run_kwargs or {}))
    if _results_hook is not None:
        _results_hook(res)
    out = np.zeros((B, L, D), np.float32)
    for c in range(NCORES):
        out[c // 4] += res.results[c]["out_p"].astype(np.float32)
    return out
